# revision 10
# baseline (speedup 1.0000x reference)
"""Trainium2 Bass kernel for the word2vec-style embedding lookup problem.

reference:
    inputs = paragraph_matrix[doc_ids] + sum(word_matrix[context_ids], axis=1)
    out_cols = outputs[:, sample_ids].transpose(1, 0, 2)
    return einsum("bd,bds->bs", inputs, out_cols)

Strategy: data-parallel over the batch dim across 8 NeuronCores. All gathers
use the Q7 `dma_gather` instruction (int16 indices over <=32768-row source
views, output position == stream position), so the host splits every gather
stream into 25000-row table chunks and sorts within chunks:

Phase A (inputs vectors):
  A1: gather the sorted-unique doc/context rows per chunk into a compact
      DRAM table (<=20480 rows, so int16-addressable as one view).
  A2: per batch tile, gather the 9 rows per batch element from the compact
      table in original order, reduce on DVE -> inputs[2048, 128] in DRAM.
Phase B (sample dots):
  Sort the 2048*16 (batch, sample) pairs by (table chunk, batch). Per chunk,
  two ALIGNED gathers: the sample row (outputs_T chunk view) and the batch's
  inputs row (2048-row inputs table). mul + reduce per stream position gives
  the dot products in stream order; the host unpermutes.

All stream lengths are padded to fixed bounds so a single SPMD program runs
on all 8 cores. `outputs` is transposed host-side so every gather is a
contiguous 512B row.
"""

import numpy as np

import concourse.mybir as mybir
from concourse.bacc import Bacc
from concourse.tile import TileContext

# Problem constants (hardcoded per harness contract).
VEC = 128
N_DOCS = 100000
N_WORDS = 100000
B = 16384
CTX = 8
NS = 16
N_CORES = 8
P = 128

B_CORE = B // N_CORES            # 2048
N_TILES = B_CORE // P            # 16
DW_ROWS = N_DOCS + N_WORDS       # doc+context id space (table rows 0..200000)
OUT_BASE = DW_ROWS               # outputs_T rows at [200000, 300000)
TABLE_ROWS = DW_ROWS + N_WORDS

CH = 25000                       # table chunk width (int16-addressable)
N_CH_A = DW_ROWS // CH           # 8
N_CH_B = N_WORDS // CH           # 4

# Per-chunk padded unique-row bounds (x128): chunks 0-3 are the doc table
# (2048 draws -> ~510 unique each), chunks 4-7 the word table (16384 draws
# -> ~3800 unique each).
A1_BOUNDS = [768] * 4 + [4224] * 4
A1_OFFS = [sum(A1_BOUNDS[:k]) for k in range(N_CH_A)]
COMPACT_ROWS = sum(A1_BOUNDS)    # 19968 (< 32768)
A2_TPG = 2                       # batch tiles per A2 gather
N_A2 = N_TILES // A2_TPG         # 8 gathers of 2*9*128 = 2304 idxs

B_BOUND = 8704                   # padded (b,s) pairs per B-chunk (x128)
SEG = 4352                       # B segment = 1 gather pair (2 per chunk)
N_SEG = N_CH_B * B_BOUND // SEG  # 8
RES_LEN = N_CH_B * B_BOUND       # 34816

IDX_COLS = (COMPACT_ROWS + N_A2 * A2_TPG * 9 * P + 2 * N_SEG * SEG) // 16


def _wrap16(stream: np.ndarray) -> np.ndarray:
    """dma_gather index layout: j at [16k + j%16, j//16], replicated 8x."""
    assert len(stream) % 16 == 0
    arr = stream.astype(np.int16).reshape(-1, 16).T  # [16, n/16]
    return np.tile(arr, (8, 1))                      # [128, n/16]


def prep_core(doc: np.ndarray, ctx: np.ndarray, smp: np.ndarray):
    """Build the per-core int16 idx tensor + the result unpermute map."""
    # ---- Phase A: doc+ctx ids in [0, 200000) ----
    ids9 = np.concatenate([doc[:, None], ctx + N_DOCS], axis=1).ravel()
    uq, inv = np.unique(ids9, return_inverse=True)
    comp_pos = np.empty(len(uq), dtype=np.int64)
    a1_streams = []
    for k in range(N_CH_A):
        lo = np.searchsorted(uq, k * CH)
        hi = np.searchsorted(uq, (k + 1) * CH)
        n = hi - lo
        assert n <= A1_BOUNDS[k], f"A1 chunk {k} unique count {n} > {A1_BOUNDS[k]}"
        comp_pos[lo:hi] = A1_OFFS[k] + np.arange(n)
        st = np.zeros(A1_BOUNDS[k], dtype=np.int64)
        st[:n] = uq[lo:hi] - k * CH
        a1_streams.append(st)
    comp_ids = comp_pos[inv].reshape(B_CORE, 9)

    a2_streams = []
    for g in range(N_A2):
        st = np.empty(A2_TPG * 9 * P, dtype=np.int64)
        for i in range(A2_TPG):
            t = g * A2_TPG + i
            # position (i*9 + r)*128 + p  ->  comp id of (batch t*128+p, role r)
            st[i * 9 * P:(i + 1) * 9 * P] = comp_ids[t * P:(t + 1) * P].T.ravel()
        a2_streams.append(st)

    # ---- Phase B: sample ids, sorted by (chunk, batch) ----
    bidx = np.repeat(np.arange(B_CORE), NS)
    w = smp.ravel().astype(np.int64)
    chk = w // CH
    order = np.lexsort((bidx, chk))
    pos_of_pair = np.empty(B_CORE * NS, dtype=np.int64)
    b_smp_streams, b_inp_streams = [], []
    for k in range(N_CH_B):
        sel = order[chk[order] == k]
        n = len(sel)
        assert n <= B_BOUND, f"B chunk {k} pair count {n} > {B_BOUND}"
        smp_st = np.zeros(B_BOUND, dtype=np.int64)
        smp_st[:n] = w[sel] - k * CH
        inp_st = np.zeros(B_BOUND, dtype=np.int64)
        inp_st[:n] = bidx[sel]
        pos_of_pair[sel] = k * B_BOUND + np.arange(n)
        b_smp_streams.append(smp_st)
        b_inp_streams.append(inp_st)

    # ---- pack all streams into one [128, IDX_COLS] int16 tensor ----
    cols = []
    for st in a1_streams + a2_streams:
        cols.append(_wrap16(st))
    for k in range(N_CH_B):
        for s in range(B_BOUND // SEG):
            cols.append(_wrap16(b_smp_streams[k][s * SEG:(s + 1) * SEG]))
            cols.append(_wrap16(b_inp_streams[k][s * SEG:(s + 1) * SEG]))
    idx = np.concatenate(cols, axis=1)
    assert idx.shape == (P, IDX_COLS), idx.shape
    return idx, pos_of_pair


def build_nc():
    nc = Bacc("TRN2")
    f32, i16 = mybir.dt.float32, mybir.dt.int16
    table = nc.dram_tensor("table", [TABLE_ROWS, VEC], f32, kind="ExternalInput")
    idx = nc.dram_tensor("idx", [P, IDX_COLS], i16, kind="ExternalInput")
    res = nc.dram_tensor("res", [RES_LEN], f32, kind="ExternalOutput")
    compact = nc.dram_tensor("compact", [COMPACT_ROWS, VEC], f32, kind="Internal")
    inputs_d = nc.dram_tensor("inputs_d", [B_CORE, VEC], f32, kind="Internal")

    with TileContext(nc) as tc:
        with (
            tc.tile_pool(name="idxp", bufs=1) as idx_pool,
            tc.tile_pool(name="a1", bufs=2) as a1_pool,
            tc.tile_pool(name="a2", bufs=3) as a2_pool,
            tc.tile_pool(name="bp", bufs=2) as b_pool,
            tc.tile_pool(name="acc", bufs=1) as acc_pool,
        ):
            idx_all = idx_pool.tile([P, IDX_COLS], mybir.dt.int16)
            nc.sync.dma_start(out=idx_all[:, :], in_=idx[:, :])

            col = 0  # running idx column offset (int16 cols = stream/16)

            def idx_slice(n):
                nonlocal col
                ap = idx_all[:, col:col + n // 16]
                col += n // 16
                return ap

            # ---- A1: unique doc/ctx rows -> compact table ----
            for k in range(N_CH_A):
                bnd = A1_BOUNDS[k]
                stg = a1_pool.tile([P, bnd // P, VEC], mybir.dt.float32,
                                   tag="a1stg")
                nc.gpsimd.dma_gather(
                    stg[:, :, :],
                    table[k * CH:(k + 1) * CH, :],
                    idx_slice(bnd),
                    bnd, bnd, VEC,
                    queue_num=0, single_packet=False,
                )
                nc.sync.dma_start(
                    out=compact[A1_OFFS[k]:A1_OFFS[k] + bnd, :].rearrange(
                        "(a p) d -> p a d", p=P),
                    in_=stg[:, :, :],
                )

            # ---- A2: per-tile 9-row gather + sum -> inputs ----
            inputs_all = acc_pool.tile([P, N_TILES, VEC], mybir.dt.float32)
            for g in range(N_A2):
                gt = a2_pool.tile([P, A2_TPG * 9, VEC], mybir.dt.float32)
                nc.gpsimd.dma_gather(
                    gt[:, :, :],
                    compact[:, :],
                    idx_slice(A2_TPG * 9 * P),
                    A2_TPG * 9 * P, A2_TPG * 9 * P, VEC,
                    queue_num=0, single_packet=False,
                )
                for i in range(A2_TPG):
                    t = g * A2_TPG + i
                    nc.vector.reduce_sum(
                        out=inputs_all[:, t, :],
                        in_=gt[:, i * 9:(i + 1) * 9, :].transpose([0, 2, 1]),
                        axis=mybir.AxisListType.X,
                    )
            nc.sync.dma_start(
                out=inputs_d[:, :].rearrange("(t p) d -> p t d", p=P),
                in_=inputs_all[:, :, :],
            )

            # ---- B: aligned sample-row + inputs-row gathers, mul, reduce ----
            res_all = acc_pool.tile([P, N_SEG, SEG // P], mybir.dt.float32)
            for seg in range(N_SEG):
                k = seg // (B_BOUND // SEG)
                smp_t = b_pool.tile([P, SEG // P, VEC], mybir.dt.float32,
                                    tag="smp")
                inp_t = b_pool.tile([P, SEG // P, VEC], mybir.dt.float32,
                                    tag="inp")
                nc.gpsimd.dma_gather(
                    smp_t[:, :, :],
                    table[OUT_BASE + k * CH:OUT_BASE + (k + 1) * CH, :],
                    idx_slice(SEG),
                    SEG, SEG, VEC,
                    queue_num=0, single_packet=False,
                )
                nc.gpsimd.dma_gather(
                    inp_t[:, :, :],
                    inputs_d[:, :],
                    idx_slice(SEG),
                    SEG, SEG, VEC,
                    queue_num=0, single_packet=False,
                )
                nc.vector.tensor_mul(
                    out=smp_t[:, :, :], in0=smp_t[:, :, :], in1=inp_t[:, :, :])
                nc.vector.reduce_sum(
                    out=res_all[:, seg, :],
                    in_=smp_t[:, :, :],
                    axis=mybir.AxisListType.X,
                )
            nc.sync.dma_start(
                out=res[:].rearrange("(a p) -> p a", p=P),
                in_=res_all[:, :, :].rearrange("p s a -> p (s a)"),
            )

    nc.finalize()
    return nc


def prepare_host(doc_ids, context_ids, sample_ids, paragraph_matrix,
                 word_matrix, outputs):
    doc_ids = np.asarray(doc_ids).astype(np.int64)
    context_ids = np.asarray(context_ids).astype(np.int64)
    sample_ids = np.asarray(sample_ids).astype(np.int64)
    table = np.concatenate(
        [
            np.asarray(paragraph_matrix, dtype=np.float32),
            np.asarray(word_matrix, dtype=np.float32),
            np.ascontiguousarray(np.asarray(outputs, dtype=np.float32).T),
        ],
        axis=0,
    )
    cores = []
    for c in range(N_CORES):
        sl = slice(c * B_CORE, (c + 1) * B_CORE)
        idx, pos = prep_core(doc_ids[sl], context_ids[sl], sample_ids[sl])
        cores.append((idx, pos))
    return table, cores


def kernel(doc_ids, context_ids, sample_ids, paragraph_matrix, word_matrix,
           outputs):
    from concourse.bass_utils import run_bass_kernel_spmd

    table, cores = prepare_host(doc_ids, context_ids, sample_ids,
                                paragraph_matrix, word_matrix, outputs)
    nc = build_nc()
    in_maps = [{"table": table, "idx": idx} for idx, _ in cores]
    out = run_bass_kernel_spmd(nc, in_maps, core_ids=list(range(N_CORES)))

    result = np.empty((B, NS), dtype=np.float32)
    for c, (_, pos) in enumerate(cores):
        flat = out.results[c]["res"][pos]  # stream order -> pair order
        result[c * B_CORE:(c + 1) * B_CORE] = flat.reshape(B_CORE, NS)
    return result


if __name__ == "__main__":
    pass


# revision 11
# speedup vs baseline: 1.9220x; 1.9220x over previous
"""Trainium2 Bass kernel for the word2vec-style embedding lookup problem.

reference:
    inputs = paragraph_matrix[doc_ids] + sum(word_matrix[context_ids], axis=1)
    out_cols = outputs[:, sample_ids].transpose(1, 0, 2)
    return einsum("bd,bds->bs", inputs, out_cols)

Strategy: data-parallel over the batch dim across 8 NeuronCores. All gathers
use the Q7 `dma_gather` instruction (int16 indices over <=32768-row source
views, output position == stream position), so the host splits every gather
stream into 25000-row table chunks and sorts within chunks:

Phase A (inputs vectors):
  A1: gather the sorted-unique doc/context rows per chunk into a compact
      DRAM table (<=20480 rows, so int16-addressable as one view).
  A2: per batch tile, gather the 9 rows per batch element from the compact
      table in original order, reduce on DVE -> inputs[2048, 128] in DRAM.
Phase B (sample dots):
  Sort the 2048*16 (batch, sample) pairs by (table chunk, batch). Per chunk,
  two ALIGNED gathers: the sample row (outputs_T chunk view) and the batch's
  inputs row (2048-row inputs table). mul + reduce per stream position gives
  the dot products in stream order; the host unpermutes.

All stream lengths are padded to fixed bounds so a single SPMD program runs
on all 8 cores. `outputs` is transposed host-side so every gather is a
contiguous 512B row.
"""

import numpy as np

import concourse.mybir as mybir
from concourse.bacc import Bacc
from concourse.tile import TileContext

# Problem constants (hardcoded per harness contract).
VEC = 128
N_DOCS = 100000
N_WORDS = 100000
B = 16384
CTX = 8
NS = 16
N_CORES = 8
P = 128

B_CORE = B // N_CORES            # 2048
N_TILES = B_CORE // P            # 16
DW_ROWS = N_DOCS + N_WORDS       # doc+context id space (table rows 0..200000)
OUT_BASE = DW_ROWS               # outputs_T rows at [200000, 300000)
TABLE_ROWS = DW_ROWS + N_WORDS

CH = 25000                       # table chunk width (int16-addressable)
N_CH_A = DW_ROWS // CH           # 8
N_CH_B = N_WORDS // CH           # 4

# Per-chunk padded unique-row bounds (x128): chunks 0-3 are the doc table
# (2048 draws -> ~510 unique each), chunks 4-7 the word table (16384 draws
# -> ~3800 unique each).
A1_BOUNDS = [768] * 4 + [4224] * 4
A1_OFFS = [sum(A1_BOUNDS[:k]) for k in range(N_CH_A)]
COMPACT_ROWS = sum(A1_BOUNDS)    # 19968 (< 32768)
A2_TPG = 2                       # batch tiles per A2 gather
N_A2 = N_TILES // A2_TPG         # 8 gathers of 2*9*128 = 2304 idxs

B_BOUND = 8704                   # padded (b,s) pairs per B-chunk (x128)
SEG = 4352                       # B segment = 1 gather pair (2 per chunk)
N_SEG = N_CH_B * B_BOUND // SEG  # 8
RES_LEN = N_CH_B * B_BOUND       # 34816

IDX_COLS = (COMPACT_ROWS + N_A2 * A2_TPG * 9 * P + 2 * N_SEG * SEG) // 16


def _wrap16(stream: np.ndarray) -> np.ndarray:
    """dma_gather index layout: j at [16k + j%16, j//16], replicated 8x."""
    assert len(stream) % 16 == 0
    arr = stream.astype(np.int16).reshape(-1, 16).T  # [16, n/16]
    return np.tile(arr, (8, 1))                      # [128, n/16]


def prep_core(doc: np.ndarray, ctx: np.ndarray, smp: np.ndarray):
    """Build the per-core int16 idx tensor + the result unpermute map."""
    # ---- Phase A: doc+ctx ids in [0, 200000) ----
    ids9 = np.concatenate([doc[:, None], ctx + N_DOCS], axis=1).ravel()
    uq, inv = np.unique(ids9, return_inverse=True)
    comp_pos = np.empty(len(uq), dtype=np.int64)
    a1_streams = []
    for k in range(N_CH_A):
        lo = np.searchsorted(uq, k * CH)
        hi = np.searchsorted(uq, (k + 1) * CH)
        n = hi - lo
        assert n <= A1_BOUNDS[k], f"A1 chunk {k} unique count {n} > {A1_BOUNDS[k]}"
        comp_pos[lo:hi] = A1_OFFS[k] + np.arange(n)
        st = np.zeros(A1_BOUNDS[k], dtype=np.int64)
        st[:n] = uq[lo:hi] - k * CH
        a1_streams.append(st)
    comp_ids = comp_pos[inv].reshape(B_CORE, 9)

    a2_streams = []
    for g in range(N_A2):
        st = np.empty(A2_TPG * 9 * P, dtype=np.int64)
        for i in range(A2_TPG):
            t = g * A2_TPG + i
            # position (i*9 + r)*128 + p  ->  comp id of (batch t*128+p, role r)
            st[i * 9 * P:(i + 1) * 9 * P] = comp_ids[t * P:(t + 1) * P].T.ravel()
        a2_streams.append(st)

    # ---- Phase B: sample ids, sorted by (chunk, batch) ----
    bidx = np.repeat(np.arange(B_CORE), NS)
    w = smp.ravel().astype(np.int64)
    chk = w // CH
    order = np.lexsort((bidx, chk))
    pos_of_pair = np.empty(B_CORE * NS, dtype=np.int64)
    b_smp_streams, b_inp_streams = [], []
    for k in range(N_CH_B):
        sel = order[chk[order] == k]
        n = len(sel)
        assert n <= B_BOUND, f"B chunk {k} pair count {n} > {B_BOUND}"
        smp_st = np.zeros(B_BOUND, dtype=np.int64)
        smp_st[:n] = w[sel] - k * CH
        inp_st = np.zeros(B_BOUND, dtype=np.int64)
        inp_st[:n] = bidx[sel]
        pos_of_pair[sel] = k * B_BOUND + np.arange(n)
        b_smp_streams.append(smp_st)
        b_inp_streams.append(inp_st)

    # ---- pack all streams into one [128, IDX_COLS] int16 tensor ----
    cols = []
    for st in a1_streams + a2_streams:
        cols.append(_wrap16(st))
    for k in range(N_CH_B):
        for s in range(B_BOUND // SEG):
            cols.append(_wrap16(b_smp_streams[k][s * SEG:(s + 1) * SEG]))
            cols.append(_wrap16(b_inp_streams[k][s * SEG:(s + 1) * SEG]))
    idx = np.concatenate(cols, axis=1)
    assert idx.shape == (P, IDX_COLS), idx.shape
    return idx, pos_of_pair


def build_nc(queue_map=None):
    nc = Bacc("TRN2", num_swdge_queues=4)
    f32, i16 = mybir.dt.float32, mybir.dt.int16
    table = nc.dram_tensor("table", [TABLE_ROWS, VEC], f32, kind="ExternalInput")
    idx = nc.dram_tensor("idx", [P, IDX_COLS], i16, kind="ExternalInput")
    res = nc.dram_tensor("res", [RES_LEN], f32, kind="ExternalOutput")
    compact = nc.dram_tensor("compact", [COMPACT_ROWS, VEC], f32, kind="Internal")
    inputs_d = nc.dram_tensor("inputs_d", [B_CORE, VEC], f32, kind="Internal")

    qi = [0]

    def next_q():
        q = queue_map[qi[0]] if queue_map is not None else 0
        qi[0] += 1
        return q

    with TileContext(nc) as tc:
        with (
            tc.tile_pool(name="idxp", bufs=1) as idx_pool,
            tc.tile_pool(name="a1", bufs=2) as a1_pool,
            tc.tile_pool(name="a2", bufs=3) as a2_pool,
            tc.tile_pool(name="bp", bufs=2) as b_pool,
            tc.tile_pool(name="acc", bufs=1) as acc_pool,
        ):
            idx_all = idx_pool.tile([P, IDX_COLS], mybir.dt.int16)
            nc.sync.dma_start(out=idx_all[:, :], in_=idx[:, :])

            col = 0  # running idx column offset (int16 cols = stream/16)

            def idx_slice(n):
                nonlocal col
                ap = idx_all[:, col:col + n // 16]
                col += n // 16
                return ap

            # ---- A1: unique doc/ctx rows -> compact table ----
            for k in range(N_CH_A):
                bnd = A1_BOUNDS[k]
                stg = a1_pool.tile([P, bnd // P, VEC], mybir.dt.float32,
                                   tag="a1stg")
                nc.gpsimd.dma_gather(
                    stg[:, :, :],
                    table[k * CH:(k + 1) * CH, :],
                    idx_slice(bnd),
                    bnd, bnd, VEC,
                    queue_num=next_q(), single_packet=False,
                )
                nc.sync.dma_start(
                    out=compact[A1_OFFS[k]:A1_OFFS[k] + bnd, :].rearrange(
                        "(a p) d -> p a d", p=P),
                    in_=stg[:, :, :],
                )

            # ---- A2: per-tile 9-row gather + sum -> inputs ----
            inputs_all = acc_pool.tile([P, N_TILES, VEC], mybir.dt.float32)
            for g in range(N_A2):
                gt = a2_pool.tile([P, A2_TPG * 9, VEC], mybir.dt.float32)
                nc.gpsimd.dma_gather(
                    gt[:, :, :],
                    compact[:, :],
                    idx_slice(A2_TPG * 9 * P),
                    A2_TPG * 9 * P, A2_TPG * 9 * P, VEC,
                    queue_num=next_q(), single_packet=False,
                )
                for i in range(A2_TPG):
                    t = g * A2_TPG + i
                    nc.vector.reduce_sum(
                        out=inputs_all[:, t, :],
                        in_=gt[:, i * 9:(i + 1) * 9, :].transpose([0, 2, 1]),
                        axis=mybir.AxisListType.X,
                    )
            nc.sync.dma_start(
                out=inputs_d[:, :].rearrange("(t p) d -> p t d", p=P),
                in_=inputs_all[:, :, :],
            )

            # ---- B: aligned sample-row + inputs-row gathers, mul, reduce ----
            res_all = acc_pool.tile([P, N_SEG, SEG // P], mybir.dt.float32)
            for seg in range(N_SEG):
                k = seg // (B_BOUND // SEG)
                smp_t = b_pool.tile([P, SEG // P, VEC], mybir.dt.float32,
                                    tag="smp")
                inp_t = b_pool.tile([P, SEG // P, VEC], mybir.dt.float32,
                                    tag="inp")
                nc.gpsimd.dma_gather(
                    smp_t[:, :, :],
                    table[OUT_BASE + k * CH:OUT_BASE + (k + 1) * CH, :],
                    idx_slice(SEG),
                    SEG, SEG, VEC,
                    queue_num=next_q(), single_packet=False,
                )
                nc.gpsimd.dma_gather(
                    inp_t[:, :, :],
                    inputs_d[:, :],
                    idx_slice(SEG),
                    SEG, SEG, VEC,
                    queue_num=next_q(), single_packet=False,
                )
                nc.vector.tensor_mul(
                    out=smp_t[:, :, :], in0=smp_t[:, :, :], in1=inp_t[:, :, :])
                nc.vector.reduce_sum(
                    out=res_all[:, seg, :],
                    in_=smp_t[:, :, :],
                    axis=mybir.AxisListType.X,
                )
            nc.sync.dma_start(
                out=res[:].rearrange("(a p) -> p a", p=P),
                in_=res_all[:, :, :].rearrange("p s a -> p (s a)"),
            )

    nc.finalize()
    return nc


def prepare_host(doc_ids, context_ids, sample_ids, paragraph_matrix,
                 word_matrix, outputs):
    doc_ids = np.asarray(doc_ids).astype(np.int64)
    context_ids = np.asarray(context_ids).astype(np.int64)
    sample_ids = np.asarray(sample_ids).astype(np.int64)
    table = np.concatenate(
        [
            np.asarray(paragraph_matrix, dtype=np.float32),
            np.asarray(word_matrix, dtype=np.float32),
            np.ascontiguousarray(np.asarray(outputs, dtype=np.float32).T),
        ],
        axis=0,
    )
    cores = []
    for c in range(N_CORES):
        sl = slice(c * B_CORE, (c + 1) * B_CORE)
        idx, pos = prep_core(doc_ids[sl], context_ids[sl], sample_ids[sl])
        cores.append((idx, pos))
    return table, cores


def gather_queue_map(nc):
    """Read each dma_gather's Tile-assigned DMASW lane; queue = lane % 4
    keeps every sem lane on a single SWDGE queue."""
    lanes = []
    for f in nc.m.functions:
        for blk in f.blocks:
            for ins in blk.instructions:
                if type(ins).__name__ == "InstDMAGatherAnt":
                    si = ins.sync_info
                    lane = None
                    for u in (si.on_update or []):
                        name = u.ant_name or ""
                        if name.startswith("DMASW"):
                            lane = int(name[5:].split("_")[0])
                    lanes.append((ins.name, lane))
    # instruction names I-k are in emission order; sort by numeric id
    lanes.sort(key=lambda t: int(t[0].split("-")[1]))
    return [(l % 4 if l is not None else 0) for _, l in lanes]


def build_nc_queued():
    nc0 = build_nc()
    qmap = gather_queue_map(nc0)
    return build_nc(queue_map=qmap)


def kernel(doc_ids, context_ids, sample_ids, paragraph_matrix, word_matrix,
           outputs):
    from concourse.bass_utils import run_bass_kernel_spmd

    table, cores = prepare_host(doc_ids, context_ids, sample_ids,
                                paragraph_matrix, word_matrix, outputs)
    nc = build_nc_queued()
    in_maps = [{"table": table, "idx": idx} for idx, _ in cores]
    out = run_bass_kernel_spmd(nc, in_maps, core_ids=list(range(N_CORES)))

    result = np.empty((B, NS), dtype=np.float32)
    for c, (_, pos) in enumerate(cores):
        flat = out.results[c]["res"][pos]  # stream order -> pair order
        result[c * B_CORE:(c + 1) * B_CORE] = flat.reshape(B_CORE, NS)
    return result


if __name__ == "__main__":
    pass


# revision 14
# speedup vs baseline: 13.1189x; 6.8256x over previous
"""Trainium2 Bass kernel for the word2vec-style embedding lookup problem.

reference:
    inputs = paragraph_matrix[doc_ids] + sum(word_matrix[context_ids], axis=1)
    out_cols = outputs[:, sample_ids].transpose(1, 0, 2)
    return einsum("bd,bds->bs", inputs, out_cols)

Strategy: data-parallel over the batch dim across 8 NeuronCores. All gathers
use the Q7 `dma_gather` instruction (int16 indices over <=32768-row source
views, output position == stream position), so the host splits every gather
stream into 25000-row table chunks and sorts within chunks:

Phase A (inputs vectors):
  A1: gather the sorted-unique doc/context rows per chunk into a compact
      DRAM table (<=20480 rows, so int16-addressable as one view).
  A2: per batch tile, gather the 9 rows per batch element from the compact
      table in original order, reduce on DVE -> inputs[2048, 128] in DRAM.
Phase B (sample dots):
  Sort the 2048*16 (batch, sample) pairs by (table chunk, batch). Per chunk,
  two ALIGNED gathers: the sample row (outputs_T chunk view) and the batch's
  inputs row (2048-row inputs table). mul + reduce per stream position gives
  the dot products in stream order; the host unpermutes.

All stream lengths are padded to fixed bounds so a single SPMD program runs
on all 8 cores. `outputs` is transposed host-side so every gather is a
contiguous 512B row.
"""

import numpy as np

import concourse.mybir as mybir
from concourse.bacc import Bacc
from concourse.tile import TileContext

# Problem constants (hardcoded per harness contract).
VEC = 128
N_DOCS = 100000
N_WORDS = 100000
B = 16384
CTX = 8
NS = 16
N_CORES = 8
P = 128

B_CORE = B // N_CORES            # 2048
N_TILES = B_CORE // P            # 16
DW_ROWS = N_DOCS + N_WORDS       # doc+context id space (table rows 0..200000)
OUT_BASE = DW_ROWS               # outputs_T rows at [200000, 300000)
TABLE_ROWS = DW_ROWS + N_WORDS

CH = 25000                       # table chunk width (int16-addressable)
N_CH_A = DW_ROWS // CH           # 8
N_CH_B = N_WORDS // CH           # 4

# Per-chunk padded unique-row bounds (x128): chunks 0-3 are the doc table
# (2048 draws -> ~510 unique each), chunks 4-7 the word table (16384 draws
# -> ~3800 unique each).
A1_BOUNDS = [640] * 4 + [4096] * 4
A1_OFFS = [sum(A1_BOUNDS[:k]) for k in range(N_CH_A)]
COMPACT_ROWS = sum(A1_BOUNDS)    # 19968 (< 32768)
A2_TPG = 2                       # batch tiles per A2 gather
N_A2 = N_TILES // A2_TPG         # 8 gathers of 2*9*128 = 2304 idxs

B_BOUND = 8704                   # padded (b,s) pairs per B-chunk (x128)
SEG = 4352                       # B segment = 1 gather pair (2 per chunk)
N_SEG = N_CH_B * B_BOUND // SEG  # 8
RES_LEN = N_CH_B * B_BOUND       # 34816

IDX_COLS = (COMPACT_ROWS + N_A2 * A2_TPG * 9 * P + 2 * N_SEG * SEG) // 16


def _wrap16(stream: np.ndarray) -> np.ndarray:
    """dma_gather index layout: j at [16k + j%16, j//16], replicated 8x."""
    assert len(stream) % 16 == 0
    arr = stream.astype(np.int16).reshape(-1, 16).T  # [16, n/16]
    return np.tile(arr, (8, 1))                      # [128, n/16]


def prep_core(doc: np.ndarray, ctx: np.ndarray, smp: np.ndarray):
    """Build the per-core int16 idx tensor + the result unpermute map."""
    # ---- Phase A: doc+ctx ids in [0, 200000) ----
    ids9 = np.concatenate([doc[:, None], ctx + N_DOCS], axis=1).ravel()
    uq, inv = np.unique(ids9, return_inverse=True)
    comp_pos = np.empty(len(uq), dtype=np.int64)
    a1_streams = []
    for k in range(N_CH_A):
        lo = np.searchsorted(uq, k * CH)
        hi = np.searchsorted(uq, (k + 1) * CH)
        n = hi - lo
        assert n <= A1_BOUNDS[k], f"A1 chunk {k} unique count {n} > {A1_BOUNDS[k]}"
        comp_pos[lo:hi] = A1_OFFS[k] + np.arange(n)
        st = np.zeros(A1_BOUNDS[k], dtype=np.int64)
        st[:n] = uq[lo:hi] - k * CH
        a1_streams.append(st)
    comp_ids = comp_pos[inv].reshape(B_CORE, 9)

    a2_streams = []
    for g in range(N_A2):
        st = np.empty(A2_TPG * 9 * P, dtype=np.int64)
        for i in range(A2_TPG):
            t = g * A2_TPG + i
            # position (i*9 + r)*128 + p  ->  comp id of (batch t*128+p, role r)
            st[i * 9 * P:(i + 1) * 9 * P] = comp_ids[t * P:(t + 1) * P].T.ravel()
        a2_streams.append(st)

    # ---- Phase B: sample ids, sorted by (chunk, batch) ----
    bidx = np.repeat(np.arange(B_CORE), NS)
    w = smp.ravel().astype(np.int64)
    chk = w // CH
    order = np.lexsort((bidx, chk))
    pos_of_pair = np.empty(B_CORE * NS, dtype=np.int64)
    b_smp_streams, b_inp_streams = [], []
    for k in range(N_CH_B):
        sel = order[chk[order] == k]
        n = len(sel)
        assert n <= B_BOUND, f"B chunk {k} pair count {n} > {B_BOUND}"
        smp_st = np.zeros(B_BOUND, dtype=np.int64)
        smp_st[:n] = w[sel] - k * CH
        inp_st = np.zeros(B_BOUND, dtype=np.int64)
        inp_st[:n] = bidx[sel]
        pos_of_pair[sel] = k * B_BOUND + np.arange(n)
        b_smp_streams.append(smp_st)
        b_inp_streams.append(inp_st)

    # ---- pack all streams into one [128, IDX_COLS] int16 tensor ----
    cols = []
    for st in a1_streams + a2_streams:
        cols.append(_wrap16(st))
    for k in range(N_CH_B):
        for s in range(B_BOUND // SEG):
            cols.append(_wrap16(b_smp_streams[k][s * SEG:(s + 1) * SEG]))
            cols.append(_wrap16(b_inp_streams[k][s * SEG:(s + 1) * SEG]))
    idx = np.concatenate(cols, axis=1)
    assert idx.shape == (P, IDX_COLS), idx.shape
    return idx, pos_of_pair


def build_nc(queue_map=None, reps=1):
    nc = Bacc("TRN2", num_swdge_queues=4)
    f32, i16 = mybir.dt.float32, mybir.dt.int16
    table = nc.dram_tensor("table", [TABLE_ROWS, VEC], f32, kind="ExternalInput")
    idx = nc.dram_tensor("idx", [P, IDX_COLS], i16, kind="ExternalInput")
    res = nc.dram_tensor("res", [RES_LEN], f32, kind="ExternalOutput")
    compact = nc.dram_tensor("compact", [COMPACT_ROWS, VEC], f32, kind="Internal")
    inputs_d = nc.dram_tensor("inputs_d", [B_CORE, VEC], f32, kind="Internal")

    qi = [0]

    def next_q():
        q = queue_map[qi[0] % len(queue_map)] if queue_map is not None else 0
        qi[0] += 1
        return q

    def emit_body(tc, idx_all, pools):
        idx_pool, a1_pool, a2_pool, b_pool, acc_pool = pools
        col = [0]

        def idx_slice(n):
            ap = idx_all[:, col[0]:col[0] + n // 16]
            col[0] += n // 16
            return ap

        # ---- A1: unique doc/ctx rows -> compact table ----
        for k in range(N_CH_A):
            bnd = A1_BOUNDS[k]
            stg = a1_pool.tile([P, bnd // P, VEC], mybir.dt.float32,
                               tag="a1stg")
            nc.gpsimd.dma_gather(
                stg[:, :, :],
                table[k * CH:(k + 1) * CH, :],
                idx_slice(bnd),
                bnd, bnd, VEC,
                queue_num=next_q(), single_packet=False,
            )
            nc.sync.dma_start(
                out=compact[A1_OFFS[k]:A1_OFFS[k] + bnd, :].rearrange(
                    "(a p) d -> p a d", p=P),
                in_=stg[:, :, :],
            )

        # ---- A2: per-tile 9-row gather + sum -> inputs ----
        inputs_all = acc_pool.tile([P, N_TILES, VEC], mybir.dt.float32,
                                   tag="inp_all")
        for g in range(N_A2):
            gt = a2_pool.tile([P, A2_TPG * 9, VEC], mybir.dt.float32,
                              tag="a2g")
            nc.gpsimd.dma_gather(
                gt[:, :, :],
                compact[:, :],
                idx_slice(A2_TPG * 9 * P),
                A2_TPG * 9 * P, A2_TPG * 9 * P, VEC,
                queue_num=next_q(), single_packet=False,
            )
            for i in range(A2_TPG):
                t = g * A2_TPG + i
                nc.vector.reduce_sum(
                    out=inputs_all[:, t, :],
                    in_=gt[:, i * 9:(i + 1) * 9, :].transpose([0, 2, 1]),
                    axis=mybir.AxisListType.X,
                )
        nc.sync.dma_start(
            out=inputs_d[:, :].rearrange("(t p) d -> p t d", p=P),
            in_=inputs_all[:, :, :],
        )

        # ---- B: aligned sample-row + inputs-row gathers, mul, reduce ----
        res_all = acc_pool.tile([P, N_SEG, SEG // P], mybir.dt.float32,
                                tag="res_all")
        for seg in range(N_SEG):
            k = seg // (B_BOUND // SEG)
            smp_t = b_pool.tile([P, SEG // P, VEC], mybir.dt.float32,
                                tag="smp")
            inp_t = b_pool.tile([P, SEG // P, VEC], mybir.dt.float32,
                                tag="inp")
            nc.gpsimd.dma_gather(
                smp_t[:, :, :],
                table[OUT_BASE + k * CH:OUT_BASE + (k + 1) * CH, :],
                idx_slice(SEG),
                SEG, SEG, VEC,
                queue_num=next_q(), single_packet=False,
            )
            nc.gpsimd.dma_gather(
                inp_t[:, :, :],
                inputs_d[:, :],
                idx_slice(SEG),
                SEG, SEG, VEC,
                queue_num=next_q(), single_packet=False,
            )
            nc.vector.tensor_mul(
                out=smp_t[:, :, :], in0=smp_t[:, :, :], in1=inp_t[:, :, :])
            nc.vector.reduce_sum(
                out=res_all[:, seg, :],
                in_=smp_t[:, :, :],
                axis=mybir.AxisListType.X,
            )
        nc.sync.dma_start(
            out=res[:].rearrange("(a p) -> p a", p=P),
            in_=res_all[:, :, :].rearrange("p s a -> p (s a)"),
        )

    with TileContext(nc) as tc:
        with (
            tc.tile_pool(name="idxp", bufs=1) as idx_pool,
            tc.tile_pool(name="a1", bufs=2) as a1_pool,
            tc.tile_pool(name="a2", bufs=3) as a2_pool,
            tc.tile_pool(name="bp", bufs=3) as b_pool,
            tc.tile_pool(name="acc", bufs=1) as acc_pool,
        ):
            idx_all = idx_pool.tile([P, IDX_COLS], mybir.dt.int16)
            nc.sync.dma_start(out=idx_all[:, :], in_=idx[:, :])
            pools = (idx_pool, a1_pool, a2_pool, b_pool, acc_pool)
            for _rep in range(reps):
                emit_body(tc, idx_all, pools)

    # Run the Bacc compile pipeline (register allocation, wait splitting via
    # event semaphores) -- run_bass_via_pjrt expects a finalized module.
    nc.finalize()
    return nc


def prepare_host(doc_ids, context_ids, sample_ids, paragraph_matrix,
                 word_matrix, outputs):
    doc_ids = np.asarray(doc_ids).astype(np.int64)
    context_ids = np.asarray(context_ids).astype(np.int64)
    sample_ids = np.asarray(sample_ids).astype(np.int64)
    table = np.concatenate(
        [
            np.asarray(paragraph_matrix, dtype=np.float32),
            np.asarray(word_matrix, dtype=np.float32),
            np.ascontiguousarray(np.asarray(outputs, dtype=np.float32).T),
        ],
        axis=0,
    )
    cores = []
    for c in range(N_CORES):
        sl = slice(c * B_CORE, (c + 1) * B_CORE)
        idx, pos = prep_core(doc_ids[sl], context_ids[sl], sample_ids[sl])
        cores.append((idx, pos))
    return table, cores


def gather_queue_map(nc):
    """Read each dma_gather's Tile-assigned DMASW lane; queue = lane % 4
    keeps every sem lane on a single SWDGE queue."""
    lanes = []
    for f in nc.m.functions:
        for blk in f.blocks:
            for ins in blk.instructions:
                if type(ins).__name__ == "InstDMAGatherAnt":
                    si = ins.sync_info
                    lane = None
                    for u in (si.on_update or []):
                        name = u.ant_name or ""
                        if name.startswith("DMASW"):
                            lane = int(name[5:].split("_")[0])
                    lanes.append((ins.name, lane))
    # instruction names I-k are in emission order; sort by numeric id
    lanes.sort(key=lambda t: int(t[0].split("-")[1]))
    return [(l % 4 if l is not None else 0) for _, l in lanes]


def build_nc_queued(reps=1):
    nc0 = build_nc(reps=reps)
    qmap = gather_queue_map(nc0)
    nc1 = build_nc(queue_map=qmap, reps=reps)
    qmap1 = gather_queue_map(nc1)
    if qmap1 != qmap:
        nc1 = build_nc(queue_map=qmap1, reps=reps)
    return nc1


def kernel(doc_ids, context_ids, sample_ids, paragraph_matrix, word_matrix,
           outputs):
    from concourse.bass_utils import run_bass_kernel_spmd

    table, cores = prepare_host(doc_ids, context_ids, sample_ids,
                                paragraph_matrix, word_matrix, outputs)
    nc = build_nc_queued()
    in_maps = [{"table": table, "idx": idx} for idx, _ in cores]
    out = run_bass_kernel_spmd(nc, in_maps, core_ids=list(range(N_CORES)))

    result = np.empty((B, NS), dtype=np.float32)
    for c, (_, pos) in enumerate(cores):
        flat = out.results[c]["res"][pos]  # stream order -> pair order
        result[c * B_CORE:(c + 1) * B_CORE] = flat.reshape(B_CORE, NS)
    return result


if __name__ == "__main__":
    pass
